# revision 1
# baseline (speedup 1.0000x reference)
"""DiT block kernel for Trainium2, 8-way data parallel (one batch element per core).

Layout strategy per core (batch element): token-major residual stream x (128-token
tiles on partitions, features on free dim) for LayerNorm stats and residual adds;
feature-major ("transposed") activations for all matmuls, produced by PE-transpose
with the DiT modulate (scale/shift) fused into the PSUM->SBUF copyback. Attention
uses transposed scores (keys on partitions) so softmax needs no max-subtraction
(scores are bounded, exp is safe in fp32) and normalization is deferred past the
attn@v matmul via an appended ones-column on V (the softmax denominator comes out
as an extra output row). All matmuls run in float32r (fp32 data, fast PE mode).
"""

import sys
from contextlib import ExitStack

for _p in ("/opt/trn_rl_repo",):
    if _p not in sys.path:
        sys.path.append(_p)

import numpy as np

import concourse.bass as bass
import concourse.mybir as mybir
import concourse.tile as tile
from concourse import bacc
from concourse.bass_utils import run_bass_kernel_spmd
from concourse.masks import make_identity

F32 = mybir.dt.float32
F32R = mybir.dt.float32r
AF = mybir.ActivationFunctionType
ALU = mybir.AluOpType

L, C, H, D, FF = 1024, 1152, 16, 72, 4608
P = 128
LT = L // P  # 8 token tiles
CT = C // P  # 9 feature blocks
FT = FF // P  # 36 ff blocks
EPS = 1e-6
SCL = float(D) ** -0.5
CH = C // 2  # 576, half of the feature dim for half-width weight tiles


def _mm(nc, out, lhsT, rhs, start, stop):
    nc.tensor.matmul(out, lhsT, rhs, start=start, stop=stop)


def build_nc():
    nc = bacc.Bacc(None, target_bir_lowering=False, debug=False)

    x_in = nc.declare_dram_parameter("x", [L, C], F32, isOutput=False)
    c_in = nc.declare_dram_parameter("c", [C], F32, isOutput=False)
    w_qkv = nc.declare_dram_parameter("W_qkv", [C, 3 * C], F32, isOutput=False)
    b_qkv = nc.declare_dram_parameter("b_qkv", [3 * C], F32, isOutput=False)
    w_proj = nc.declare_dram_parameter("W_proj", [C, C], F32, isOutput=False)
    b_proj = nc.declare_dram_parameter("b_proj", [C], F32, isOutput=False)
    w_fc1 = nc.declare_dram_parameter("W_fc1", [C, FF], F32, isOutput=False)
    b_fc1 = nc.declare_dram_parameter("b_fc1", [FF], F32, isOutput=False)
    w_fc2 = nc.declare_dram_parameter("W_fc2", [FF, C], F32, isOutput=False)
    b_fc2 = nc.declare_dram_parameter("b_fc2", [C], F32, isOutput=False)
    w_ada = nc.declare_dram_parameter("W_ada", [C, 6 * C], F32, isOutput=False)
    b_ada = nc.declare_dram_parameter("b_ada", [6 * C], F32, isOutput=False)
    out = nc.declare_dram_parameter("out", [L, C], F32, isOutput=True)

    x_r = x_in.rearrange("(i p) c -> i p c", p=P)
    out_r = out.rearrange("(i p) c -> i p c", p=P)
    wqkv_r = w_qkv.rearrange("(k p) n -> p k n", p=P)
    wproj_r = w_proj.rearrange("(k p) n -> p k n", p=P)
    wfc1_r = w_fc1.rearrange("(k p) n -> p k n", p=P)
    wfc2_r = w_fc2.rearrange("(k p) n -> p k n", p=P)

    with tile.TileContext(nc) as tc:
        import os as _os
        for _rep in range(int(_os.environ.get("DIT_REPS", "1"))):
            with (
                tc.tile_pool(name=f"dram{_rep}", bufs=1, space="DRAM") as dram,
                tc.tile_pool(name=f"const{_rep}", bufs=1) as const,
                tc.tile_pool(name=f"xp{_rep}", bufs=1) as xp,
                tc.tile_pool(name=f"pv{_rep}", bufs=1) as pvp,
                tc.tile_pool(name=f"hTs{_rep}", bufs=1) as hTs,
                tc.tile_pool(name=f"wbig{_rep}", bufs=1) as wbig,
                tc.tile_pool(name=f"bcp{_rep}", bufs=2) as bcp,
                tc.tile_pool(name=f"stats{_rep}", bufs=12) as stats,
            ):
                qkT_hbm = dram.tile([2 * C, L], F32)
                oT_hbm = dram.tile([C, L], F32)
                ada_hbm = dram.tile([1, 6 * C], F32)

                identity = const.tile([P, P], F32)
                make_identity(nc, identity)
                ones = const.tile([1, P], F32R)
                nc.scalar.activation(ones[:], nc.const_aps.tensor(1.0, (1, P)), AF.Identity)
                adaT = const.tile([P, 54], F32)
                sm1_msa = const.tile([P, CT], F32)
                sm1_mlp = const.tile([P, CT], F32)
                bqkv_fm = const.tile([P, 2 * CT], F32)
                bfc1_fm = const.tile([P, FT], F32)
                cT = const.tile([P, CT], F32)
                siluc = const.tile([P, CT], F32R)
                eps_t = const.tile([P, 1], F32)
                nc.gpsimd.memset(eps_t[:], EPS)

                x_sb = xp.tile([P, LT, C], F32)
                # v_aug and y_sb have disjoint lifetimes -> same slot
                v_aug = pvp.tile([P, LT, H, D + 1], F32R, tag="pv", name="v_aug")

                nc.sync.dma_start(bqkv_fm[:], b_qkv[0 : 2 * C].rearrange("(j p) -> p j", p=P))
                nc.sync.dma_start(bfc1_fm[:], b_fc1.rearrange("(j p) -> p j", p=P))
                for i in range(LT):
                    nc.sync.dma_start(x_sb[:, i, :], x_r[i])
                nc.sync.dma_start(cT[:], c_in.rearrange("(k p) -> p k", p=P))
                nc.scalar.activation(siluc[:], cT[:], AF.Silu)

                def bcast(tc, psum_pool, dst, src_row_ap, nm):
                    # replicate a (1,C) row onto all 128 partitions: ones outer-product
                    row = tc_rows.tile([1, C], F32R, tag="bc_row", name=f"bcrow_{nm}", bufs=1)
                    nc.sync.dma_start(row[:], src_row_ap.bitcast(F32R))
                    for n3 in range(3):
                        ps = psum_pool.tile([P, 384], F32, tag="bc", name=f"bc_ps_{nm}_{n3}")
                        sl = slice(n3 * 384, (n3 + 1) * 384)
                        _mm(nc, ps[:], ones[:], row[:, sl], True, True)
                        nc.scalar.copy(dst[:, sl], ps[:])

                def ada_chunks(psum_pool, wpool, rpool, n_lo, n_hi):
                    # ada[chunk] = silu(c) @ W_ada[:, chunk] + b_ada[chunk]
                    for n in range(n_lo, n_hi):
                        c0 = n * 512
                        w = min(512, 6 * C - c0)
                        ps = psum_pool.tile([1, 512], F32, tag="ada", name=f"ada_ps_{n}")
                        for k in range(CT):
                            wt = wpool.tile([P, 512], F32R, tag="wada", name=f"wada_{n}_{k}")
                            nc.sync.dma_start(
                                wt[:, :w], w_ada[k * P : (k + 1) * P, c0 : c0 + w].bitcast(F32R)
                            )
                            _mm(nc, ps[:, :w], siluc[:, k : k + 1], wt[:, :w], k == 0, k == CT - 1)
                        brow = rpool.tile([1, 512], F32, tag="brow", name=f"bada_{n}")
                        nc.sync.dma_start(
                            brow[:, :w],
                            b_ada[c0 : c0 + w].rearrange("(o n) -> o n", o=1),
                        )
                        row = rpool.tile([1, 512], F32, tag="ada_row", name=f"ada_row_{n}")
                        nc.vector.tensor_add(row[:, :w], ps[:, :w], brow[:, :w])
                        nc.sync.dma_start(ada_hbm[0:1, c0 : c0 + w], row[:, :w])

                def layernorm_transpose(scpool, trps, i, dst, shift_col0, sm1):
                    xi = x_sb[:, i, :]
                    s1 = stats.tile([P, 1], F32, tag="st", name=f"s1_{i}")
                    nc.vector.tensor_reduce(s1[:], xi, mybir.AxisListType.X, ALU.add)
                    xsq = scpool.tile([P, C], F32, tag="sc1152", name=f"xsq_{i}")
                    s2 = stats.tile([P, 1], F32, tag="st", name=f"s2_{i}")
                    nc.scalar.activation(xsq[:], xi, AF.Square, accum_out=s2[:])
                    mu = stats.tile([P, 1], F32, tag="st", name=f"mu_{i}")
                    nc.vector.tensor_scalar_mul(mu[:], s1[:], 1.0 / C)
                    ex2 = stats.tile([P, 1], F32, tag="st", name=f"ex2_{i}")
                    nc.vector.tensor_scalar_mul(ex2[:], s2[:], 1.0 / C)
                    nmu2 = stats.tile([P, 1], F32, tag="st", name=f"nmu2_{i}")
                    nc.vector.scalar_tensor_tensor(
                        nmu2[:], mu[:], -1.0, mu[:], ALU.mult, ALU.mult
                    )
                    var = stats.tile([P, 1], F32, tag="st", name=f"var_{i}")
                    nc.vector.tensor_add(var[:], ex2[:], nmu2[:])
                    sd = stats.tile([P, 1], F32, tag="st", name=f"sd_{i}")
                    nc.scalar.activation(sd[:], var[:], AF.Sqrt, bias=eps_t[:])
                    rs = stats.tile([P, 1], F32, tag="st", name=f"rs_{i}")
                    nc.vector.reciprocal(rs[:], sd[:])
                    nmr = stats.tile([P, 1], F32, tag="st", name=f"nmr_{i}")
                    nc.vector.scalar_tensor_tensor(
                        nmr[:], mu[:], -1.0, rs[:], ALU.mult, ALU.mult
                    )
                    xhat = scpool.tile([P, C], F32, tag="sc1152", name=f"xhat_{i}")
                    nc.scalar.activation(xhat[:], xi, AF.Identity, bias=nmr[:], scale=rs[:])
                    for j in range(CT):
                        tp = trps.tile([P, P], F32, tag="tr", name=f"tr_{i}_{j}")
                        nc.tensor.transpose(tp[:], xhat[:, j * P : (j + 1) * P], identity[:])
                        nc.scalar.activation(
                            dst[:, j, i * P : (i + 1) * P],
                            tp[:],
                            AF.Identity,
                            bias=adaT[:, shift_col0 + j : shift_col0 + j + 1],
                            scale=sm1[:, j : j + 1],
                        )

                # ======== phase A: ada (msa half) + LN1 ========
                hT = hTs.tile([P, CT, L], F32R, tag="hT", name="hT1")
                with (
                    tc.tile_pool(name=f"rows{_rep}", bufs=2) as tc_rows,
                    tc.tile_pool(name=f"wada_p{_rep}", bufs=3) as wada_p,
                    tc.tile_pool(name=f"sc1{_rep}", bufs=3) as sc1,
                    tc.tile_pool(name=f"ada_ps{_rep}", bufs=2, space="PSUM") as ada_ps,
                    tc.tile_pool(name=f"tr_ps1{_rep}", bufs=2, space="PSUM") as tr_ps1,
                ):
                    ada_chunks(ada_ps, wada_p, tc_rows, 0, 5)
                    nc.sync.dma_start(
                        adaT[:, 0:20], ada_hbm[0, 0:2560].rearrange("(g p) -> p g", p=P)
                    )
                    nc.vector.tensor_scalar_add(sm1_msa[:], adaT[:, 9:18], 1.0)
                    for i in range(LT):
                        layernorm_transpose(sc1, tr_ps1, i, hT, 0, sm1_msa)

                # ======== phase B: v = h @ W_v (token-major, ones column added) ========
                nc.scalar.activation(
                    v_aug[:, :, :, D : D + 1],
                    nc.const_aps.tensor(1.0, (P, LT, H, 1)),
                    AF.Identity,
                )
                with tc.tile_pool(name=f"rows2{_rep}", bufs=2) as tc_rows:
                    with tc.tile_pool(name=f"bv_ps{_rep}", bufs=2, space="PSUM") as bv_ps:
                        bv_bc = bcp.tile([P, C], F32, tag="bc", name="bv_bc")
                        bcast(tc, bv_ps, bv_bc, b_qkv[2 * C : 3 * C].rearrange("(o n) -> o n", o=1), "bv")
                    v_ps = mlp_b = ExitStack()
                    v_ps = mlp_b.enter_context(tc.tile_pool(name=f"v_ps{_rep}", bufs=4, space="PSUM"))
                    for half in range(2):
                        wv = wbig.tile([P, CT, CH], F32R, tag="wbig", name=f"wv_{half}")
                        nc.sync.dma_start(
                            wv[:],
                            wqkv_r[:, :, 2 * C + half * CH : 2 * C + (half + 1) * CH].bitcast(F32R),
                        )
                        for i in range(LT):
                            pss = [
                                v_ps.tile([P, 288], F32, tag="vps", name=f"v_ps_{half}_{i}_{n}")
                                for n in range(2)
                            ]
                            for k in range(CT):
                                lhsT = hT[:, k, i * P : (i + 1) * P]
                                for n in range(2):
                                    _mm(
                                        nc,
                                        pss[n][:],
                                        lhsT,
                                        wv[:, k, n * 288 : (n + 1) * 288],
                                        k == 0,
                                        k == CT - 1,
                                    )
                            for n in range(2):
                                n4 = half * 2 + n
                                nc.vector.tensor_add(
                                    v_aug[:, i, n4 * 4 : (n4 + 1) * 4, 0:D],
                                    pss[n][:].rearrange("p (h d) -> p h d", d=D),
                                    bv_bc[:, n4 * 288 : (n4 + 1) * 288].rearrange(
                                        "p (h d) -> p h d", d=D
                                    ),
                                )

                    mlp_b.close()
                # ======== phase C: q^T,k^T feature-major -> DRAM ========
                with (
                    tc.tile_pool(name=f"wqk_p{_rep}", bufs=2) as wqk_p,
                    tc.tile_pool(name=f"qkb_p{_rep}", bufs=2) as qkb_p,
                    tc.tile_pool(name=f"qk_ps{_rep}", bufs=4, space="PSUM") as qk_ps,
                ):
                    for jb in range(2 * CT):
                        wt = wqk_p.tile([P, CT, P], F32R, tag="wqk", name=f"wqk_{jb}")
                        nc.sync.dma_start(wt[:], wqkv_r[:, :, jb * P : (jb + 1) * P].bitcast(F32R))
                        pss = [
                            qk_ps.tile([P, 512], F32, tag="qkps", name=f"qk_ps_{jb}_{n2}")
                            for n2 in range(2)
                        ]
                        for k in range(CT):
                            for n2 in range(2):
                                _mm(
                                    nc,
                                    pss[n2][:],
                                    wt[:, k, :],
                                    hT[:, k, n2 * 512 : (n2 + 1) * 512],
                                    k == 0,
                                    k == CT - 1,
                                )
                        qkb = qkb_p.tile([P, L], F32, tag="qkb", name=f"qkb_{jb}")
                        for n2 in range(2):
                            nc.scalar.activation(
                                qkb[:, n2 * 512 : (n2 + 1) * 512],
                                pss[n2][:],
                                AF.Identity,
                                bias=bqkv_fm[:, jb : jb + 1],
                            )
                        nc.sync.dma_start(qkT_hbm[jb * P : (jb + 1) * P, :], qkb[:])

                # ======== ada (mlp half) + remaining adaT ========
                with (
                    tc.tile_pool(name=f"rows2b{_rep}", bufs=2) as tc_rows,
                    tc.tile_pool(name=f"wada2{_rep}", bufs=3) as wada2_p,
                    tc.tile_pool(name=f"ada_ps2{_rep}", bufs=2, space="PSUM") as ada_ps2,
                ):
                    ada_chunks(ada_ps2, wada2_p, tc_rows, 5, 14)
                    nc.sync.dma_start(
                        adaT[:, 20:54], ada_hbm[0, 2560 : 6 * C].rearrange("(g p) -> p g", p=P)
                    )
                    nc.vector.tensor_scalar_add(sm1_mlp[:], adaT[:, 36:45], 1.0)

                # ======== phase D: attention, head at a time ========
                with (
                    tc.tile_pool(name=f"qkh{_rep}", bufs=5) as qkh,
                    tc.tile_pool(name=f"eTp{_rep}", bufs=3) as eTp,
                    tc.tile_pool(name=f"zp{_rep}", bufs=4) as zp,
                    tc.tile_pool(name=f"op{_rep}", bufs=4) as op_pool,
                    tc.tile_pool(name=f"s_ps{_rep}", bufs=4, space="PSUM") as s_ps,
                    tc.tile_pool(name=f"o_ps{_rep}", bufs=4, space="PSUM") as o_ps,
                ):
                    for hp in range(H // 2):
                        hpair = (2 * hp, 2 * hp + 1)
                        qTs, kTs, psos = {}, {}, {}
                        for h in hpair:
                            qT = qkh.tile([D, L], F32R, tag="qkh", name=f"qT_{h}")
                            nc.sync.dma_start(qT[:], qkT_hbm[h * D : (h + 1) * D, :].bitcast(F32R))
                            kT = qkh.tile([D, L], F32R, tag="qkh", name=f"kT_{h}")
                            nc.sync.dma_start(kT[:], qkT_hbm[C + h * D : C + (h + 1) * D, :].bitcast(F32R))
                            qTs[h], kTs[h] = qT, kT
                            psos[h] = [
                                o_ps.tile([D + 1, 512], F32, tag="ops", name=f"o_ps_{h}_{n2}")
                                for n2 in range(2)
                            ]
                        for m in range(LT):
                            eTs = {}
                            for h in hpair:
                                pss = [
                                    s_ps.tile([P, 512], F32, tag="sps", name=f"s_ps_{h}_{m}_{n2}")
                                    for n2 in range(2)
                                ]
                                for n2 in range(2):
                                    _mm(
                                        nc,
                                        pss[n2][:],
                                        kTs[h][:, m * P : (m + 1) * P],
                                        qTs[h][:, n2 * 512 : (n2 + 1) * 512],
                                        True,
                                        True,
                                    )
                                eT = eTp.tile([P, L], F32R, tag="eT", name=f"eT_{h}_{m}")
                                for n2 in range(2):
                                    nc.scalar.activation(
                                        eT[:, n2 * 512 : (n2 + 1) * 512],
                                        pss[n2][:],
                                        AF.Exp,
                                        scale=SCL,
                                    )
                                eTs[h] = eT
                            for h in hpair:
                                for n2 in range(2):
                                    _mm(
                                        nc,
                                        psos[h][n2][:],
                                        v_aug[:, m, h, :],
                                        eTs[h][:, n2 * 512 : (n2 + 1) * 512],
                                        m == 0,
                                        m == LT - 1,
                                    )
                        for h in hpair:
                            pso = psos[h]
                            o_sb = zp.tile([D + 1, L], F32, tag="osb", name=f"osb_{h}", bufs=2)
                            for n2 in range(2):
                                nc.scalar.copy(
                                    o_sb[:, n2 * 512 : (n2 + 1) * 512], pso[n2][:]
                                )
                            z_row = zp.tile([1, L], F32, tag="z", name=f"z_{h}", bufs=2)
                            nc.sync.dma_start(z_row[:], o_sb[D : D + 1, :])
                            rz = zp.tile([1, L], F32R, tag="z", name=f"rz_{h}", bufs=2)
                            with nc.allow_low_precision(reason="f32r is fp32-width"):
                                nc.vector.reciprocal(rz[:], z_row[:])
                            rzb = op_pool.tile([D, L], F32, tag="rzb", name=f"rzb_{h}", bufs=2)
                            oT = op_pool.tile([D, L], F32, tag="oT", name=f"oT_{h}", bufs=2)
                            for n2 in range(2):
                                sl = slice(n2 * 512, (n2 + 1) * 512)
                                psr = s_ps.tile([D, 512], F32, tag="sps", name=f"rz_ps_{h}_{n2}")
                                _mm(nc, psr[:], ones[:, 0:D], rz[:, sl], True, True)
                                nc.scalar.copy(rzb[:, sl], psr[:])
                                nc.vector.tensor_mul(oT[:, sl], o_sb[0:D, sl], rzb[:, sl])
                            nc.sync.dma_start(oT_hbm[h * D : (h + 1) * D, :], oT[:])

                # ======== phase E: proj + gated residual into x_sb ========
                oTb = hTs.tile([P, CT, L], F32R, tag="hT", name="oTb")
                nc.sync.dma_start(oTb[:], oT_hbm[:].rearrange("(k p) l -> p k l", p=P).bitcast(F32R))
                with (
                    tc.tile_pool(name=f"rows3{_rep}", bufs=2) as tc_rows,
                    tc.tile_pool(name=f"resp{_rep}", bufs=2) as resp,
                    tc.tile_pool(name=f"bc_ps3{_rep}", bufs=2, space="PSUM") as bc_ps3,
                    tc.tile_pool(name=f"pj_ps{_rep}", bufs=4, space="PSUM") as pj_ps,
                ):
                    gmsa_bc = bcp.tile([P, C], F32, tag="bc", name="gmsa_bc")
                    bcast(tc, bc_ps3, gmsa_bc, ada_hbm[0:1, 2 * C : 3 * C], "gmsa")
                    bproj_bc = bcp.tile([P, C], F32, tag="bc", name="bproj_bc")
                    bcast(tc, bc_ps3, bproj_bc, b_proj.rearrange("(o n) -> o n", o=1), "bproj")
                    for half in range(2):
                        wp = wbig.tile([P, CT, CH], F32R, tag="wbig", name=f"wproj_{half}")
                        nc.sync.dma_start(wp[:], wproj_r[:, :, half * CH : (half + 1) * CH].bitcast(F32R))
                        for i in range(LT):
                            pss = [
                                pj_ps.tile([P, 288], F32, tag="pjps", name=f"pj_ps_{half}_{i}_{n}")
                                for n in range(2)
                            ]
                            for k in range(CT):
                                lhsT = oTb[:, k, i * P : (i + 1) * P]
                                for n in range(2):
                                    _mm(
                                        nc,
                                        pss[n][:],
                                        lhsT,
                                        wp[:, k, n * 288 : (n + 1) * 288],
                                        k == 0,
                                        k == CT - 1,
                                    )
                            for n in range(2):
                                n4 = half * 2 + n
                                sl = slice(n4 * 288, (n4 + 1) * 288)
                                t = resp.tile([P, 288], F32, tag="res", name=f"res_{half}_{i}_{n}")
                                nc.vector.tensor_add(t[:], pss[n][:], bproj_bc[:, sl])
                                nc.vector.tensor_mul(t[:], t[:], gmsa_bc[:, sl])
                                nc.vector.tensor_add(x_sb[:, i, sl], x_sb[:, i, sl], t[:])

                # ======== phase F: LN2 + modulate + transpose ========
                h2T = hTs.tile([P, CT, L], F32R, tag="hT", name="h2T")
                with (
                    tc.tile_pool(name=f"sc2{_rep}", bufs=3) as sc2,
                    tc.tile_pool(name=f"tr_ps2{_rep}", bufs=2, space="PSUM") as tr_ps2,
                ):
                    for i in range(LT):
                        layernorm_transpose(sc2, tr_ps2, i, h2T, 27, sm1_mlp)

                # ======== phase G: MLP (4 chunks of FF) + final residual ========
                y_sb = pvp.tile([P, LT, C], F32, tag="pv", name="y_sb")
                NCH = 4
                JPC = FT // NCH  # 9
                with (
                    tc.tile_pool(name=f"rows4{_rep}", bufs=2) as tc_rows,
                    tc.tile_pool(name=f"wf1_p{_rep}", bufs=2) as wf1_p,
                    tc.tile_pool(name=f"uTs{_rep}", bufs=1) as uTs,
                    tc.tile_pool(name=f"resp2{_rep}", bufs=2) as resp2,
                ):
                    with tc.tile_pool(name=f"bc_ps4{_rep}", bufs=2, space="PSUM") as bc_ps4:
                        bfc2_bc = bcp.tile([P, C], F32, tag="bc", name="bfc2_bc")
                        bcast(tc, bc_ps4, bfc2_bc, b_fc2.rearrange("(o n) -> o n", o=1), "bfc2")
                        gmlp_bc = bcp.tile([P, C], F32, tag="bc", name="gmlp_bc")
                        bcast(tc, bc_ps4, gmlp_bc, ada_hbm[0:1, 5 * C : 6 * C], "gmlp")
                    mlp_ps = ExitStack()
                    f1_ps = mlp_ps.enter_context(tc.tile_pool(name=f"f1_ps{_rep}", bufs=4, space="PSUM"))
                    f2_ps = mlp_ps.enter_context(tc.tile_pool(name=f"f2_ps{_rep}", bufs=4, space="PSUM"))
                    for ch in range(NCH):
                        uT = uTs.tile([P, JPC, L], F32R, tag="uT", name=f"uT_{ch}")
                        for jj in range(JPC):
                            j = ch * JPC + jj
                            wt = wf1_p.tile([P, CT, P], F32R, tag="wf1", name=f"wfc1_{j}")
                            nc.sync.dma_start(wt[:], wfc1_r[:, :, j * P : (j + 1) * P].bitcast(F32R))
                            pss = [
                                f1_ps.tile([P, 512], F32, tag="f1ps", name=f"f1_ps_{j}_{n2}")
                                for n2 in range(2)
                            ]
                            for k in range(CT):
                                for n2 in range(2):
                                    _mm(
                                        nc,
                                        pss[n2][:],
                                        wt[:, k, :],
                                        h2T[:, k, n2 * 512 : (n2 + 1) * 512],
                                        k == 0,
                                        k == CT - 1,
                                    )
                            for n2 in range(2):
                                nc.scalar.activation(
                                    uT[:, jj, n2 * 512 : (n2 + 1) * 512],
                                    pss[n2][:],
                                    AF.Gelu,
                                    bias=bfc1_fm[:, j : j + 1],
                                )
                        for half in range(2):
                            wf2 = wbig.tile([P, JPC, CH], F32R, tag="wbig", name=f"wfc2_{ch}_{half}")
                            nc.sync.dma_start(
                                wf2[:],
                                wfc2_r[
                                    :, ch * JPC : (ch + 1) * JPC, half * CH : (half + 1) * CH
                                ].bitcast(F32R),
                            )
                            for i in range(LT):
                                pss = [
                                    f2_ps.tile(
                                        [P, 288], F32, tag="f2ps", name=f"f2_ps_{ch}_{half}_{i}_{n}"
                                    )
                                    for n in range(2)
                                ]
                                for kk in range(JPC):
                                    lhsT = uT[:, kk, i * P : (i + 1) * P]
                                    for n in range(2):
                                        _mm(
                                            nc,
                                            pss[n][:],
                                            lhsT,
                                            wf2[:, kk, n * 288 : (n + 1) * 288],
                                            kk == 0,
                                            kk == JPC - 1,
                                        )
                                for n in range(2):
                                    n4 = half * 2 + n
                                    sl = slice(n4 * 288, (n4 + 1) * 288)
                                    if ch == 0:
                                        nc.vector.tensor_add(
                                            y_sb[:, i, sl], pss[n][:], bfc2_bc[:, sl]
                                        )
                                    else:
                                        nc.vector.tensor_add(
                                            y_sb[:, i, sl], y_sb[:, i, sl], pss[n][:]
                                        )

                    mlp_ps.close()
                    # final gated residual + store
                    for i in range(LT):
                        for n4 in range(4):
                            sl = slice(n4 * 288, (n4 + 1) * 288)
                            t = resp2.tile([P, 288], F32, tag="res2", name=f"fres_{i}_{n4}")
                            nc.vector.tensor_mul(t[:], y_sb[:, i, sl], gmlp_bc[:, sl])
                            nc.vector.tensor_add(x_sb[:, i, sl], x_sb[:, i, sl], t[:])
                        nc.sync.dma_start(out_r[i], x_sb[:, i, :])


    nc.compile()
    return nc


_NC_CACHE = {}


def get_nc():
    if "nc" not in _NC_CACHE:
        _NC_CACHE["nc"] = build_nc()
    return _NC_CACHE["nc"]


def make_in_maps(inputs):
    B = inputs["x"].shape[0]
    shared = {
        k: np.ascontiguousarray(np.asarray(inputs[k], dtype=np.float32))
        for k in (
            "W_qkv",
            "b_qkv",
            "W_proj",
            "b_proj",
            "W_fc1",
            "b_fc1",
            "W_fc2",
            "b_fc2",
            "W_ada",
            "b_ada",
        )
    }
    in_maps = []
    for i in range(B):
        m = dict(shared)
        m["x"] = np.ascontiguousarray(np.asarray(inputs["x"][i], dtype=np.float32))
        m["c"] = np.ascontiguousarray(
            np.asarray(inputs["c"][i], dtype=np.float32).reshape(C)
        )
        in_maps.append(m)
    return in_maps


def kernel(**inputs):
    nc = get_nc()
    in_maps = make_in_maps(inputs)
    res = run_bass_kernel_spmd(nc, in_maps, list(range(len(in_maps))))
    return np.stack([r["out"] for r in res.results]).astype(np.float32)



# revision 33
# speedup vs baseline: 4.7618x; 4.7618x over previous
"""DiT block kernel for Trainium2, 8-way data parallel (one batch element per core).

Layout strategy per core (batch element): token-major residual stream x (128-token
tiles on partitions, features on free dim) held in fp32 SBUF for LayerNorm stats
and residual adds; feature-major ("transposed") activations for all matmuls,
produced by PE-transpose with the DiT modulate (scale/shift) fused into the
PSUM->SBUF copyback. All matmul operands are bf16 (weights converted host-side,
activations cast in the fused copybacks); accumulation stays fp32 in PSUM.
Attention is fully SBUF-resident: q^T/k^T live in blocked [128, 9, L] SBUF tiles
(heads that straddle a 128-partition block boundary split their score matmul into
two accumulating matmuls), softmax uses no max-subtraction (scores are bounded),
and normalization is deferred past attn@v via an appended ones-column on V (the
denominator comes out as an extra output row; its reciprocal is broadcast with a
PE outer product and applied while writing the blocked o^T tile). Large weight
streams are issued from the otherwise-idle Pool (gpsimd) queue so DMA issue
overhead does not serialize on the sync queue.
"""

import os
import sys
from contextlib import ExitStack

for _p in ("/opt/trn_rl_repo",):
    if _p not in sys.path:
        sys.path.append(_p)

import numpy as np

import concourse.bass as bass
import concourse.mybir as mybir
import concourse.tile as tile
from concourse import bacc
from concourse.bass_utils import run_bass_kernel_spmd
from concourse.masks import make_identity

F32 = mybir.dt.float32
F32R = mybir.dt.float32r
BF16 = mybir.dt.bfloat16
AF = mybir.ActivationFunctionType
ALU = mybir.AluOpType

L, C, H, D, FF = 1024, 1152, 16, 72, 4608
P = 128
LT = L // P  # 8 token tiles
CT = C // P  # 9 feature blocks
FT = FF // P  # 36 ff blocks
EPS = 1e-6
SCL = float(D) ** -0.5
CH = C // 2  # 576: half-width weight chunk for v/proj/fc2 streaming


def _mm(nc, out, lhsT, rhs, start, stop):
    nc.tensor.matmul(out, lhsT, rhs, start=start, stop=stop)


def _head_pieces(h):
    """SBUF partition pieces of head h in the blocked [128, CT, L] layout.

    Returns [(j, p0, r0, ln)]: block j, partition offset p0, offset r0 within
    the head, piece length ln. At most 2 pieces (D=72 < P=128).
    """
    f0 = h * D
    j0, p0 = f0 // P, f0 % P
    if p0 + D <= P:
        return [(j0, p0, 0, D)]
    ln0 = P - p0
    return [(j0, p0, 0, ln0), (j0 + 1, 0, ln0, D - ln0)]


def _block_parity_pieces(j, parity):
    """Partition ranges (p0, ln) of block j holding features of heads with
    h % 2 == parity."""
    res = []
    for h in range(parity, H, 2):
        for (jj, p0, _r0, ln) in _head_pieces(h):
            if jj == j:
                res.append((p0, ln))
    return res


def _silu(nc, pool, out, in_, sim_safe, nm):
    if not sim_safe:
        nc.scalar.activation(out, in_, AF.Silu)
        return
    sg = pool.tile(list(in_.shape), F32, tag="silu_tmp", name=f"sg_{nm}")
    nc.scalar.activation(sg[:], in_, AF.Sigmoid)
    nc.vector.tensor_mul(out, in_, sg[:])


def _gelu(nc, pool, out, psum, bias, sim_safe, nm):
    """out = gelu_tanh(psum + bias); bias is a per-partition AP."""
    if not sim_safe:
        nc.scalar.activation(out, psum, AF.Gelu, bias=bias)
        return
    shp = list(psum.shape)
    u = pool.tile(shp, F32, tag="gelu_u", name=f"gu_{nm}")
    nc.scalar.activation(u[:], psum, AF.Identity, bias=bias)
    u2 = pool.tile(shp, F32, tag="gelu_u2", name=f"gu2_{nm}")
    nc.scalar.activation(u2[:], u[:], AF.Square)
    u3 = pool.tile(shp, F32, tag="gelu_u3", name=f"gu3_{nm}")
    nc.vector.tensor_mul(u3[:], u2[:], u[:])
    inner = pool.tile(shp, F32, tag="gelu_in", name=f"gi_{nm}")
    nc.vector.scalar_tensor_tensor(inner[:], u3[:], 0.044715, u[:], ALU.mult, ALU.add)
    th = pool.tile(shp, F32, tag="gelu_th", name=f"gt_{nm}")
    nc.scalar.activation(th[:], inner[:], AF.Tanh, scale=0.7978845608028654)
    half = pool.tile(shp, F32, tag="gelu_h", name=f"gh_{nm}")
    nc.vector.scalar_tensor_tensor(half[:], th[:], 0.5, u[:], ALU.mult, ALU.mult)
    nc.vector.scalar_tensor_tensor(out, u[:], 0.5, half[:], ALU.mult, ALU.add)


def build_nc(reps=None, sim_safe=False):
    if reps is None:
        reps = int(os.environ.get("DIT_REPS", "1"))
    nc = bacc.Bacc(None, target_bir_lowering=False, debug=False)

    x_in = nc.declare_dram_parameter("x", [L, C], F32, isOutput=False)
    c_in = nc.declare_dram_parameter("c", [C], F32, isOutput=False)
    w_qkv = nc.declare_dram_parameter("W_qkv", [C, 3 * C], BF16, isOutput=False)
    b_qkv = nc.declare_dram_parameter("b_qkv", [3 * C], F32, isOutput=False)
    w_proj = nc.declare_dram_parameter("W_proj", [C, C], BF16, isOutput=False)
    b_proj = nc.declare_dram_parameter("b_proj", [C], F32, isOutput=False)
    w_fc1 = nc.declare_dram_parameter("W_fc1", [C, FF], BF16, isOutput=False)
    b_fc1 = nc.declare_dram_parameter("b_fc1", [FF], F32, isOutput=False)
    w_fc2 = nc.declare_dram_parameter("W_fc2", [FF, C], BF16, isOutput=False)
    b_fc2 = nc.declare_dram_parameter("b_fc2", [C], F32, isOutput=False)
    w_ada = nc.declare_dram_parameter("W_ada", [C, 6 * C], BF16, isOutput=False)
    b_ada = nc.declare_dram_parameter("b_ada", [6 * C], F32, isOutput=False)
    out = nc.declare_dram_parameter("out", [L, C], F32, isOutput=True)

    x_flat = x_in.rearrange("(i p) c -> p i c", p=P)
    out_r = out.rearrange("(i p) c -> i p c", p=P)
    wqkv_r = w_qkv.rearrange("(k p) n -> p k n", p=P)
    wproj_r = w_proj.rearrange("(k p) n -> p k n", p=P)
    wfc1_r = w_fc1.rearrange("(k p) n -> p k n", p=P)
    wfc2_r = w_fc2.rearrange("(k p) n -> p k n", p=P)
    wada_r = w_ada.rearrange("(k p) n -> p k n", p=P)

    hw_loop = int(os.environ.get("DIT_HWLOOP", "0"))
    with tile.TileContext(nc) as tc:
        if hw_loop and reps > 1:
            with tc.For_i(0, reps) as _iv:
                _emit_body(nc, tc, 0, sim_safe,
                           x_flat, c_in, wqkv_r, b_qkv, wproj_r, b_proj,
                           wfc1_r, b_fc1, wfc2_r, b_fc2, wada_r, b_ada, out_r)
        else:
            for rep in range(reps):
                _emit_body(nc, tc, rep, sim_safe,
                           x_flat, c_in, wqkv_r, b_qkv, wproj_r, b_proj,
                           wfc1_r, b_fc1, wfc2_r, b_fc2, wada_r, b_ada, out_r)

    nc.compile()
    return nc


def _emit_body(nc, tc, rep, sim_safe,
               x_flat, c_in, wqkv_r, b_qkv, wproj_r, b_proj,
               wfc1_r, b_fc1, wfc2_r, b_fc2, wada_r, b_ada, out_r):
    R = f"r{rep}"
    with ExitStack() as body:
        # Pools release in LIFO order: body-lifetime pools first, then the
        # attention-scoped (av_stk) and ada-scoped (ada_stk) pools on top.
        dram = body.enter_context(tc.tile_pool(name=f"dram{R}", bufs=1, space="DRAM"))
        const = body.enter_context(tc.tile_pool(name=f"const{R}", bufs=1))
        xp = body.enter_context(tc.tile_pool(name=f"xp{R}", bufs=1))
        hTs = body.enter_context(tc.tile_pool(name=f"hTs{R}", bufs=1))
        bcp = body.enter_context(tc.tile_pool(name=f"bcp{R}", bufs=2))
        stats = body.enter_context(tc.tile_pool(name=f"stats{R}", bufs=12))
        rows = body.enter_context(tc.tile_pool(name=f"rows{R}", bufs=1))
        wbig_stk = ExitStack()
        wbig = wbig_stk.enter_context(tc.tile_pool(name=f"wbig{R}", bufs=2))
        av_stk = ExitStack()
        vp = av_stk.enter_context(tc.tile_pool(name=f"vp{R}", bufs=1))
        qkp = av_stk.enter_context(tc.tile_pool(name=f"qkp{R}", bufs=1))
        ada_stk = ExitStack()
        wadap = ada_stk.enter_context(tc.tile_pool(name=f"wadap{R}", bufs=2))
        adarows = ada_stk.enter_context(tc.tile_pool(name=f"adarows{R}", bufs=1))

        ada_hbm = dram.tile([1, 6 * C], F32)

        identity = const.tile([P, P], BF16)
        make_identity(nc, identity)
        ones = const.tile([1, P], F32R)
        nc.scalar.activation(ones[:], nc.const_aps.tensor(1.0, (1, P)), AF.Identity)
        adaT = const.tile([P, 54], F32)
        sm1_msa = const.tile([P, CT], F32)
        sm1_mlp = const.tile([P, CT], F32)
        bqkv_fm = const.tile([P, 2 * CT], F32)
        bfc1_fm = const.tile([P, FT], F32)
        cT = const.tile([P, CT], F32)
        siluc = const.tile([P, CT], BF16)
        eps_t = const.tile([P, 1], F32)
        nc.gpsimd.memset(eps_t[:], EPS)
        zrows = const.tile([D, L], BF16)
        nc.vector.memset(zrows[:], 0.0)

        x_sb = xp.tile([P, LT, C], F32)
        v_aug = vp.tile([P, LT, H, 97], BF16, tag="pv", name="v_aug")
        # q is stored twice with the complementary heads' features zeroed, so a
        # head's score matmul can contract a full 128-partition block at base 0
        # (PE requires base partition 0 for K>64); the adjacent head's features
        # multiply zeros and vanish. k keeps one blocked copy.
        q_even = qkp.tile([P, CT, L], BF16, tag="qe", name="q_even")
        q_odd = qkp.tile([P, CT, L], BF16, tag="qo", name="q_odd")
        k_sb = qkp.tile([P, CT, L], BF16, tag="k", name="k_sb")

        nc.sync.dma_start(cT[:], c_in.rearrange("(k p) -> p k", p=P))
        _silu(nc, rows, siluc[:], cT[:], sim_safe, R)
        nc.sync.dma_start(bqkv_fm[:], b_qkv[0 : 2 * C].rearrange("(j p) -> p j", p=P))
        nc.sync.dma_start(bfc1_fm[:], b_fc1.rearrange("(j p) -> p j", p=P))
        for i in range(LT):
            nc.sync.dma_start(x_sb[:, i, :], x_flat[:, i, :])

        def bcast(psum_pool, ps_tag, dst, src_row_ap, nm):
            # replicate a (1,C) row onto all 128 partitions: ones outer-product
            row = rows.tile([1, C], F32R, tag="bc_row", name=f"bcrow_{nm}", bufs=2)
            nc.sync.dma_start(row[:], src_row_ap.bitcast(F32R))
            for n4 in range(4):
                ps = psum_pool.tile([P, 288], F32, tag=ps_tag, name=f"bc_ps_{nm}_{n4}")
                sl = slice(n4 * 288, (n4 + 1) * 288)
                _mm(nc, ps[:], ones[:], row[:, sl], True, True)
                nc.scalar.copy(dst[:, sl], ps[:])

        def ada_chunk(psum_pool, g):
            # ada[1152g:1152(g+1)] = silu(c) @ W_ada[:, chunk] + b_ada[chunk]
            c0 = g * C
            pss = [
                psum_pool.tile([1, 288], F32, tag="ada", name=f"ada_ps_{R}_{g}_{s}")
                for s in range(4)
            ]
            for half in range(2):
                wt = wadap.tile([P, CT, CH], BF16, tag="wada", name=f"wada_{R}_{g}_{half}")
                nc.gpsimd.dma_start(
                    wt[:], wada_r[:, :, c0 + half * CH : c0 + (half + 1) * CH]
                )
                for k in range(CT):
                    for s in (2 * half, 2 * half + 1):
                        _mm(nc, pss[s][:], siluc[:, k : k + 1],
                            wt[:, k, (s - 2 * half) * 288 : (s - 2 * half + 1) * 288],
                            k == 0, k == CT - 1)
            brow = adarows.tile([1, C], F32, tag="brow", name=f"bada_{R}_{g}")
            nc.sync.dma_start(brow[:], b_ada[c0 : c0 + C].rearrange("(o n) -> o n", o=1))
            for s in range(4):
                sl = slice(s * 288, (s + 1) * 288)
                nc.vector.tensor_add(brow[:, sl], pss[s][:], brow[:, sl])
            nc.sync.dma_start(ada_hbm[0:1, c0 : c0 + C], brow[:])
            nc.sync.dma_start(
                adaT[:, 9 * g : 9 * (g + 1)],
                ada_hbm[0, c0 : c0 + C].rearrange("(g p) -> p g", p=P),
            )

        def layernorm_transpose(scpool, trps, i, dst, shift_col0, sm1):
            xi = x_sb[:, i, :]
            s1 = stats.tile([P, 1], F32, tag="st", name=f"s1_{R}_{i}")
            nc.vector.tensor_reduce(s1[:], xi, mybir.AxisListType.X, ALU.add)
            xsq = scpool.tile([P, C], BF16, tag="sc1152", name=f"xsq_{R}_{i}")
            s2 = stats.tile([P, 1], F32, tag="st", name=f"s2_{R}_{i}")
            nc.scalar.activation(xsq[:], xi, AF.Square, accum_out=s2[:])
            mu = stats.tile([P, 1], F32, tag="st", name=f"mu_{R}_{i}")
            nc.vector.tensor_scalar_mul(mu[:], s1[:], 1.0 / C)
            ex2 = stats.tile([P, 1], F32, tag="st", name=f"ex2_{R}_{i}")
            nc.vector.tensor_scalar_mul(ex2[:], s2[:], 1.0 / C)
            nmu2 = stats.tile([P, 1], F32, tag="st", name=f"nmu2_{R}_{i}")
            nc.vector.scalar_tensor_tensor(nmu2[:], mu[:], -1.0, mu[:], ALU.mult, ALU.mult)
            var = stats.tile([P, 1], F32, tag="st", name=f"var_{R}_{i}")
            nc.vector.tensor_add(var[:], ex2[:], nmu2[:])
            sd = stats.tile([P, 1], F32, tag="st", name=f"sd_{R}_{i}")
            nc.scalar.activation(sd[:], var[:], AF.Sqrt, bias=eps_t[:])
            rs = stats.tile([P, 1], F32, tag="st", name=f"rs_{R}_{i}")
            nc.vector.reciprocal(rs[:], sd[:])
            nmr = stats.tile([P, 1], F32, tag="st", name=f"nmr_{R}_{i}")
            nc.vector.scalar_tensor_tensor(nmr[:], mu[:], -1.0, rs[:], ALU.mult, ALU.mult)
            xhat = scpool.tile([P, C], BF16, tag="schat", name=f"xhat_{R}_{i}")
            nc.scalar.activation(xhat[:], xi, AF.Identity, bias=nmr[:], scale=rs[:])
            for j in range(CT):
                tp = trps.tile([P, P], BF16, tag="tr", name=f"tr_{R}_{i}_{j}")
                nc.tensor.transpose(tp[:], xhat[:, j * P : (j + 1) * P], identity[:])
                nc.scalar.activation(
                    dst[:, j, i * P : (i + 1) * P],
                    tp[:],
                    AF.Identity,
                    bias=adaT[:, shift_col0 + j : shift_col0 + j + 1],
                    scale=sm1[:, j : j + 1],
                )

        # ======== phase A: ada chunks 0,1 (shift/scale msa) + LN1 ========
        hT = hTs.tile([P, CT, L], BF16, tag="hT", name=f"hT1_{R}")
        with (
            tc.tile_pool(name=f"sc1{R}", bufs=2) as sc1,
            tc.tile_pool(name=f"ada_ps{R}", bufs=4, space="PSUM") as ada_ps,
            tc.tile_pool(name=f"tr_ps1{R}", bufs=2, space="PSUM") as tr_ps1,
        ):
            ada_chunk(ada_ps, 0)
            ada_chunk(ada_ps, 1)
            nc.vector.tensor_scalar_add(sm1_msa[:], adaT[:, 9:18], 1.0)
            ada_chunk(ada_ps, 2)
            ada_chunk(ada_ps, 3)
            for i in range(LT):
                layernorm_transpose(sc1, tr_ps1, i, hT, 0, sm1_msa)
            ada_chunk(ada_ps, 4)
            ada_chunk(ada_ps, 5)
            nc.vector.tensor_scalar_add(sm1_mlp[:], adaT[:, 36:45], 1.0)
        ada_stk.close()  # free the W_ada stream buffers early

        # ======== phase B: v = h @ W_v (token-major, ones column added) ========
        # zero cols 72:96 (read by the AV matmul), ones column at 96 so the
        # softmax denominator lands at partition 96 (legal engine base).
        nc.vector.memset(v_aug[:], 0.0)
        nc.scalar.activation(
            v_aug[:, :, :, 96:97],
            nc.const_aps.tensor(1.0, (P, LT, H, 1)),
            AF.Identity,
        )
        with tc.tile_pool(name=f"v_ps{R}", bufs=4, space="PSUM") as v_ps:
            bv_bc = bcp.tile([P, C], BF16, tag="bc", name=f"bv_bc_{R}")
            bcast(v_ps, "vps", bv_bc, b_qkv[2 * C : 3 * C].rearrange("(o n) -> o n", o=1), f"bv_{R}")
            for half in range(2):
                wv = wbig.tile([P, CT, CH], BF16, tag="wbig", name=f"wv_{R}_{half}")
                nc.gpsimd.dma_start(
                    wv[:], wqkv_r[:, :, 2 * C + half * CH : 2 * C + (half + 1) * CH]
                )
                for i in range(LT):
                    pss = [
                        v_ps.tile([P, 288], F32, tag="vps", name=f"v_ps_{R}_{half}_{i}_{n}")
                        for n in range(2)
                    ]
                    for k in range(CT):
                        lhsT = hT[:, k, i * P : (i + 1) * P]
                        for n in range(2):
                            _mm(nc, pss[n][:], lhsT, wv[:, k, n * 288 : (n + 1) * 288],
                                k == 0, k == CT - 1)
                    for n in range(2):
                        n4 = half * 2 + n
                        nc.vector.tensor_add(
                            v_aug[:, i, n4 * 4 : (n4 + 1) * 4, 0:D],
                            pss[n][:].rearrange("p (h d) -> p h d", d=D),
                            bv_bc[:, n4 * 288 : (n4 + 1) * 288].rearrange("p (h d) -> p h d", d=D),
                        )

        # ======== phases C+D merged: q/k projection interleaved with attention ====
        # One attention head pair becomes ready per q/k block (pair p needs
        # blocks <= last block of head 2p+1 = p+1), so attention's Act-bound
        # softmax overlaps the remaining projection matmuls on PE.
        nc.vector.memset(q_odd[:], 0.0)
        oTb = hTs.tile([P, CT, L], BF16, tag="hT", name=f"oTb_{R}")
        with (
            tc.tile_pool(name=f"eTp{R}", bufs=2) as eTp,
            tc.tile_pool(name=f"zp{R}", bufs=4) as zp,
            tc.tile_pool(name=f"qk_ps{R}", bufs=2, space="PSUM") as qk_ps,
            tc.tile_pool(name=f"s_ps{R}", bufs=2, space="PSUM") as s_ps,
            tc.tile_pool(name=f"o_ps{R}", bufs=4, space="PSUM") as o_ps,
        ):
            def emit_qk_block(jb):
                wt = wbig.tile([P, CT, 2 * P], BF16, tag="wqkpair", name=f"wqk_{R}_{jb}", bufs=3)
                nc.gpsimd.dma_start(wt[:, :, 0:P], wqkv_r[:, :, jb * P : (jb + 1) * P])
                nc.gpsimd.dma_start(
                    wt[:, :, P : 2 * P], wqkv_r[:, :, C + jb * P : C + (jb + 1) * P]
                )
                for qk in range(2):  # 0 = q, 1 = k
                    pss = [
                        qk_ps.tile([P, 512], F32, tag="qkps", name=f"qk_ps_{R}_{qk}_{jb}_{n2}")
                        for n2 in range(2)
                    ]
                    for n2 in range(2):
                        sl = slice(n2 * 512, (n2 + 1) * 512)
                        for k in range(CT):
                            _mm(nc, pss[n2][:], wt[:, k, qk * P : (qk + 1) * P],
                                hT[:, k, sl], k == 0, k == CT - 1)
                    col = qk * CT + jb
                    dst = k_sb if qk == 1 else q_even
                    for n2 in range(2):
                        sl = slice(n2 * 512, (n2 + 1) * 512)
                        nc.vector.tensor_scalar_add(
                            dst[:, jb, sl], pss[n2][:], bqkv_fm[:, col : col + 1]
                        )

            def emit_mask(h):
                # copy head h (odd) into the pre-zeroed q_odd, then zero it in
                # q_even (DMA is the only partition-unconstrained engine)
                for (j, p0, _r0, ln) in _head_pieces(h):
                    nc.sync.dma_start(q_odd[p0 : p0 + ln, j, :], q_even[p0 : p0 + ln, j, :])
                for (j, p0, _r0, ln) in _head_pieces(h):
                    nc.sync.dma_start(q_even[p0 : p0 + ln, j, :], zrows[0:ln, :])

            def emit_pair(hp):
                hpair = (2 * hp, 2 * hp + 1)
                psos = {
                    h: [
                        o_ps.tile([97, 512], F32, tag="ops", name=f"o_ps_{R}_{h}_{n2}")
                        for n2 in range(2)
                    ]
                    for h in hpair
                }
                for m in range(LT):
                    eTs = {}
                    for h in hpair:
                        qpar = q_even if h % 2 == 0 else q_odd
                        blocks = sorted({j for (j, _, _, _) in _head_pieces(h)})
                        pss = [
                            s_ps.tile([P, 512], F32, tag="sps", name=f"s_ps_{R}_{h}_{m}_{n2}")
                            for n2 in range(2)
                        ]
                        for n2 in range(2):
                            for pi, j in enumerate(blocks):
                                _mm(
                                    nc,
                                    pss[n2][:],
                                    k_sb[:, j, m * P : (m + 1) * P],
                                    qpar[:, j, n2 * 512 : (n2 + 1) * 512],
                                    pi == 0,
                                    pi == len(blocks) - 1,
                                )
                        eT = eTp.tile([P, L], BF16, tag="eT", name=f"eT_{R}_{h}_{m}")
                        for n2 in range(2):
                            nc.scalar.activation(
                                eT[:, n2 * 512 : (n2 + 1) * 512], pss[n2][:], AF.Exp, scale=SCL
                            )
                        eTs[h] = eT
                    for h in hpair:
                        for n2 in range(2):
                            _mm(nc, psos[h][n2][:], v_aug[:, m, h, :],
                                eTs[h][:, n2 * 512 : (n2 + 1) * 512], m == 0, m == LT - 1)
                for h in hpair:
                    pso = psos[h]
                    # copy PSUM out fast on Act to free the AV banks for the
                    # next head pair; the normalize chain then runs from SBUF
                    o_sb = zp.tile([97, L], F32, tag="osb", name=f"osb_{R}_{h}", bufs=2)
                    for n2 in range(2):
                        nc.scalar.copy(o_sb[:, n2 * 512 : (n2 + 1) * 512], pso[n2][:])
                    rz = zp.tile([1, L], F32R, tag="z", name=f"rz_{R}_{h}", bufs=1)
                    with nc.allow_low_precision(reason="f32r is fp32-width"):
                        nc.vector.reciprocal(rz[:], o_sb[96:97, :])
                    o_nrm = zp.tile([D, L], BF16, tag="onrm", name=f"onrm_{R}_{h}", bufs=2)
                    for n2 in range(2):
                        sl = slice(n2 * 512, (n2 + 1) * 512)
                        psr = s_ps.tile([D, 512], F32, tag="sps", name=f"rz_ps_{R}_{h}_{n2}")
                        _mm(nc, psr[:], ones[:, 0:D], rz[:, sl], True, True)
                        rzb = zp.tile([D, 512], F32, tag="rzb", name=f"rzb_{R}_{h}_{n2}", bufs=2)
                        nc.vector.tensor_copy(rzb[:], psr[:])
                        nc.vector.tensor_mul(o_nrm[:, sl], o_sb[0:D, sl], rzb[:])
                    for (j, p0, r0, ln) in _head_pieces(h):
                        nc.sync.dma_start(
                            oTb[p0 : p0 + ln, j, :], o_nrm[r0 : r0 + ln, :]
                        )

            masked = set()
            pair_done = 0
            for jb in range(CT):
                emit_qk_block(jb)
                for h in range(1, H, 2):
                    if h not in masked and _head_pieces(h)[-1][0] <= jb:
                        emit_mask(h)
                        masked.add(h)
                while pair_done < H // 2 and _head_pieces(2 * pair_done + 1)[-1][0] <= jb:
                    emit_pair(pair_done)
                    pair_done += 1
        av_stk.close()  # free q/k/v buffers before the MLP phase

        # ======== phase E: proj + gated residual into x_sb ========
        h2T = hTs.tile([P, CT, L], BF16, tag="hT", name=f"h2T_{R}")
        with (
            tc.tile_pool(name=f"resp{R}", bufs=2) as resp,
            tc.tile_pool(name=f"sc2{R}", bufs=2) as sc2,
            tc.tile_pool(name=f"pj_ps{R}", bufs=4, space="PSUM") as pj_ps,
            tc.tile_pool(name=f"tr_ps2{R}", bufs=2, space="PSUM") as tr_ps2,
        ):
            gmsa_bc = bcp.tile([P, C], BF16, tag="bc", name=f"gmsa_bc_{R}")
            bcast(pj_ps, "pjps", gmsa_bc, ada_hbm[0:1, 2 * C : 3 * C], f"gmsa_{R}")
            bproj_bc = bcp.tile([P, C], BF16, tag="bc", name=f"bproj_bc_{R}")
            bcast(pj_ps, "pjps", bproj_bc, b_proj.rearrange("(o n) -> o n", o=1), f"bproj_{R}")
            wps = []
            for half in range(2):
                wp = wbig.tile([P, CT, CH], BF16, tag="wbig", name=f"wproj_{R}_{half}")
                nc.gpsimd.dma_start(wp[:], wproj_r[:, :, half * CH : (half + 1) * CH])
                wps.append(wp)
            for i in range(LT):
                for half in range(2):
                    pss = [
                        pj_ps.tile([P, 288], F32, tag="pjps", name=f"pj_ps_{R}_{half}_{i}_{n}")
                        for n in range(2)
                    ]
                    for k in range(CT):
                        lhsT = oTb[:, k, i * P : (i + 1) * P]
                        for n in range(2):
                            _mm(nc, pss[n][:], lhsT, wps[half][:, k, n * 288 : (n + 1) * 288],
                                k == 0, k == CT - 1)
                    for n in range(2):
                        n4 = half * 2 + n
                        sl = slice(n4 * 288, (n4 + 1) * 288)
                        t = resp.tile([P, 288], F32, tag="res", name=f"res_{R}_{half}_{i}_{n}")
                        nc.vector.tensor_add(t[:], pss[n][:], bproj_bc[:, sl])
                        nc.vector.tensor_mul(t[:], t[:], gmsa_bc[:, sl])
                        nc.vector.tensor_add(x_sb[:, i, sl], x_sb[:, i, sl], t[:])
                # LN2 for this token tile overlaps the remaining proj tiles
                layernorm_transpose(sc2, tr_ps2, i, h2T, 27, sm1_mlp)
        wbig_stk.close()  # proj was the last wbig user

        # ======== phase G: MLP (4 chunks of FF) + final residual ========
        NCH = 4
        JPC = FT // NCH  # 9 fc1 blocks per chunk
        with (
            tc.tile_pool(name=f"wf1_p{R}", bufs=2) as wf1_p,
            tc.tile_pool(name=f"wf2_p{R}", bufs=3) as wf2p,
            tc.tile_pool(name=f"ysp{R}", bufs=1) as ysp,
            tc.tile_pool(name=f"uTs{R}", bufs=1) as uTs,
            tc.tile_pool(name=f"glp{R}", bufs=1) as glp,
            tc.tile_pool(name=f"resp2{R}", bufs=2) as resp2,
            tc.tile_pool(name=f"f1_ps{R}", bufs=4, space="PSUM") as f1_ps,
            tc.tile_pool(name=f"f2_ps{R}", bufs=4, space="PSUM") as f2_ps,
        ):
            y_sb = ysp.tile([P, LT, C], F32, tag="y", name=f"y_sb_{R}")
            bfc2_bc = bcp.tile([P, C], BF16, tag="bc", name=f"bfc2_bc_{R}")
            bcast(f2_ps, "f2ps", bfc2_bc, b_fc2.rearrange("(o n) -> o n", o=1), f"bfc2_{R}")
            gmlp_bc = bcp.tile([P, C], BF16, tag="bc", name=f"gmlp_bc_{R}")
            bcast(f2_ps, "f2ps", gmlp_bc, ada_hbm[0:1, 5 * C : 6 * C], f"gmlp_{R}")
            for ch in range(NCH):
                uT = uTs.tile([P, JPC, L], BF16, tag="uT", name=f"uT_{R}_{ch}")
                # fc1 weights stream in 512-col chunks (4 j-blocks each)
                for jg in range(JPC // 4 + 1):  # 3 groups: 4,4,1 blocks
                    j0 = jg * 4
                    nj = min(4, JPC - j0)
                    if nj <= 0:
                        continue
                    wt = wf1_p.tile([P, CT, 512], BF16, tag="wf1", name=f"wfc1_{R}_{ch}_{jg}")
                    gc0 = ch * JPC * P + j0 * P
                    nc.gpsimd.dma_start(wt[:, :, 0 : nj * P], wfc1_r[:, :, gc0 : gc0 + nj * P])
                    for jj in range(nj):
                        j = ch * JPC + j0 + jj
                        pss = [
                            f1_ps.tile([P, 512], F32, tag="f1ps", name=f"f1_ps_{R}_{j}_{n2}")
                            for n2 in range(2)
                        ]
                        for k in range(CT):
                            for n2 in range(2):
                                _mm(nc, pss[n2][:], wt[:, k, jj * P : (jj + 1) * P],
                                    h2T[:, k, n2 * 512 : (n2 + 1) * 512], k == 0, k == CT - 1)
                        for n2 in range(2):
                            _gelu(
                                nc, glp,
                                uT[:, j0 + jj, n2 * 512 : (n2 + 1) * 512],
                                pss[n2][:],
                                bfc1_fm[:, j : j + 1],
                                sim_safe,
                                f"{R}_{j}_{n2}",
                            )
                wf2s = []
                for half in range(2):
                    wf2 = wf2p.tile([P, JPC, CH], BF16, tag="wf2", name=f"wfc2_{R}_{ch}_{half}")
                    nc.gpsimd.dma_start(
                        wf2[:], wfc2_r[:, ch * JPC : (ch + 1) * JPC, half * CH : (half + 1) * CH]
                    )
                    wf2s.append(wf2)
                for i in range(LT):
                    for half in range(2):
                        pss = [
                            f2_ps.tile([P, 288], F32, tag="f2ps", name=f"f2_ps_{R}_{ch}_{half}_{i}_{n}")
                            for n in range(2)
                        ]
                        for kk in range(JPC):
                            lhsT = uT[:, kk, i * P : (i + 1) * P]
                            for n in range(2):
                                _mm(nc, pss[n][:], lhsT, wf2s[half][:, kk, n * 288 : (n + 1) * 288],
                                    kk == 0, kk == JPC - 1)
                        for n in range(2):
                            n4 = half * 2 + n
                            sl = slice(n4 * 288, (n4 + 1) * 288)
                            if ch == 0:
                                nc.vector.tensor_add(y_sb[:, i, sl], pss[n][:], bfc2_bc[:, sl])
                            else:
                                nc.vector.tensor_add(y_sb[:, i, sl], y_sb[:, i, sl], pss[n][:])
                    if ch == NCH - 1:
                        # final gated residual + store for this tile
                        for n4 in range(4):
                            sl = slice(n4 * 288, (n4 + 1) * 288)
                            t = resp2.tile([P, 288], F32, tag="res2", name=f"fres_{R}_{i}_{n4}")
                            nc.vector.tensor_mul(t[:], y_sb[:, i, sl], gmlp_bc[:, sl])
                            nc.vector.tensor_add(x_sb[:, i, sl], x_sb[:, i, sl], t[:])
                        nc.sync.dma_start(out_r[i], x_sb[:, i, :])



_NC_CACHE = {}


def get_nc(reps=None, sim_safe=False):
    key = (reps, sim_safe)
    if key not in _NC_CACHE:
        _NC_CACHE[key] = build_nc(reps=reps, sim_safe=sim_safe)
    return _NC_CACHE[key]


def make_in_maps(inputs):
    from ml_dtypes import bfloat16

    B = inputs["x"].shape[0]
    shared = {}
    for k in ("W_qkv", "W_proj", "W_fc1", "W_fc2", "W_ada"):
        shared[k] = np.ascontiguousarray(
            np.asarray(inputs[k], dtype=np.float32).astype(bfloat16)
        )
    for k in ("b_qkv", "b_proj", "b_fc1", "b_fc2", "b_ada"):
        shared[k] = np.ascontiguousarray(np.asarray(inputs[k], dtype=np.float32))
    in_maps = []
    for i in range(B):
        m = dict(shared)
        m["x"] = np.ascontiguousarray(np.asarray(inputs["x"][i], dtype=np.float32))
        m["c"] = np.ascontiguousarray(np.asarray(inputs["c"][i], dtype=np.float32).reshape(C))
        in_maps.append(m)
    return in_maps


def kernel(**inputs):
    nc = get_nc()
    in_maps = make_in_maps(inputs)
    res = run_bass_kernel_spmd(nc, in_maps, list(range(len(in_maps))))
    return np.stack([r["out"] for r in res.results]).astype(np.float32)
